# revision 4
# baseline (speedup 1.0000x reference)
"""Chamfer loss (nn_ChamferLoss_45157286150461) Trainium2 Bass kernel.

Math (matches the reference):
    P[b,i,j] = ||gts[b,i]||^2 + ||preds[b,j]||^2 - 2 gts[b,i].preds[b,j]
    out = mean_j min_i P  +  mean_i min_j P       (means over all b,j / b,i)

Sharding: data-parallel over batch. 8 cores x 2 batches each. Each core
returns a single f32 partial = sum(min_i P) + sum(min_j P) over its two
batches; the host sums the 8 partials and divides by B*N.

Device-side per (batch):
  - augmented fp32 matmul on PE: lhsT = [-2x | ||x||^2 | 1] (K=5) against
    rhs = [y | 1 | ||y||^2] producing 128x512 distance tiles in PSUM.
  - ScalarE converts each PSUM tile to fp16 in SBUF.
  - VectorE: elementwise-min accumulate into M (for the min-over-i
    direction) + free-dim min reduce via tensor_scalar accum (min-over-j).
  - Epilogue: PE-transpose of M chunks + free-dim min for the partition
    direction; sums via reduce-add and a ones-matmul partition sum.
"""

import os
import sys
from contextlib import ExitStack

for _p in ("/opt/trn_rl_repo",):
    if os.path.isdir(_p) and _p not in sys.path:
        sys.path.insert(0, _p)

import numpy as np

import concourse.bass as bass  # noqa: F401
import concourse.tile as tile
from concourse import bacc, mybir
from concourse.bass_utils import run_bass_kernel_spmd

f32 = mybir.dt.float32
f16 = mybir.dt.float16
AX = mybir.AxisListType
OP = mybir.AluOpType
ACTF = mybir.ActivationFunctionType

N_CORES = 8
B = 16
N = 4096
D = 3
BPC = B // N_CORES  # batches per core
P = 128             # i-tile (PSUM partition dim)
JW = 512            # j-tile (PSUM free dim)
NIT = N // P
NJT = N // JW
BIG = 60000.0       # > any squared distance here, < fp16 max

# "ts": dr-reduce via tensor_scalar(min) with accum_out (single-src op)
# "ttr": dr-reduce via tensor_tensor_reduce (fallback if ts accum is off)
DR_MODE = os.environ.get("CHAMFER_DR_MODE", "ts")


def build_program(do_compile=True, loop_reps=None):
    nc = bacc.Bacc("TRN2", target_bir_lowering=False, debug=False)

    xt = nc.dram_tensor("xt", [BPC, D, N], f32, kind="ExternalInput")
    xr = nc.dram_tensor("xr", [BPC, NIT, D * P], f32, kind="ExternalInput")
    yt = nc.dram_tensor("yt", [BPC, D, N], f32, kind="ExternalInput")
    yr = nc.dram_tensor("yr", [BPC, NIT, D * P], f32, kind="ExternalInput")
    ones = nc.dram_tensor("ones", [1, N], f32, kind="ExternalInput")
    ident = nc.dram_tensor("ident", [P, P], f16, kind="ExternalInput")
    out = nc.dram_tensor("out", [1, 1], f32, kind="ExternalOutput")

    with ExitStack() as ctx:
        tc = ctx.enter_context(tile.TileContext(nc))
        consts = ctx.enter_context(tc.tile_pool(name="consts", bufs=1))
        augp = ctx.enter_context(tc.tile_pool(name="aug", bufs=2))
        prep = ctx.enter_context(tc.tile_pool(name="prep", bufs=2))
        tpool = ctx.enter_context(tc.tile_pool(name="tconv", bufs=4))
        mpool = ctx.enter_context(tc.tile_pool(name="mmin", bufs=2))
        accp = ctx.enter_context(tc.tile_pool(name="acc", bufs=2))
        resp = ctx.enter_context(tc.tile_pool(name="res", bufs=1))
        ps_mm = ctx.enter_context(tc.tile_pool(name="psmm", bufs=4, space="PSUM"))
        ps_tr = ctx.enter_context(tc.tile_pool(name="pstr", bufs=2, space="PSUM"))
        ps_fin = ctx.enter_context(tc.tile_pool(name="psfin", bufs=1, space="PSUM"))

        ident_sb = consts.tile([P, P], f16)
        nc.sync.dma_start(ident_sb[:], ident[:])
        ones_col = consts.tile([P, 1], f32)
        nc.vector.memset(ones_col[:], 1.0)
        res = resp.tile([1, BPC], f32)

        if loop_reps is not None:
            ctx.enter_context(tc.For_i(0, loop_reps, 1))

        for b in range(BPC):
            # ---- build augmented operands ----
            augx = augp.tile([5, N], f32, tag="augx")
            augy = augp.tile([5, N], f32, tag="augy")
            nc.sync.dma_start(augx[0:3, :], xt[b])
            nc.sync.dma_start(augx[4:5, :], ones[:])
            nc.sync.dma_start(augy[0:3, :], yt[b])
            nc.sync.dma_start(augy[3:4, :], ones[:])
            for w_dram, aug, row in ((xr, augx, 3), (yr, augy, 4)):
                w = prep.tile([NIT, D * P], f32, tag="w")
                nc.sync.dma_start(w[:], w_dram[b])
                sq = prep.tile([NIT, D * P], f32, tag="sq")
                nc.vector.tensor_mul(sq[:], w[:], w[:])
                s = prep.tile([NIT, P], f32, tag="s")
                nc.vector.tensor_reduce(
                    s[:],
                    sq[:].rearrange("p (k c) -> p k c", c=D),
                    axis=AX.X,
                    op=OP.add,
                )
                # flatten [NIT, P] -> the aug row [1, N] (SBUF->SBUF DMA)
                nc.sync.dma_start(aug[row : row + 1, :], s[:])
            nc.vector.tensor_scalar_mul(augx[0:3, :], augx[0:3, :], -2.0)

            # ---- main loop ----
            M = mpool.tile([P, N], f16, tag="M")
            DR = accp.tile([P, NIT], f32, tag="DR")
            DL = accp.tile([P, NIT], f32, tag="DL")
            for it in range(NIT):
                lhsT = augx[:, it * P : (it + 1) * P]
                drcol = accp.tile([P, NJT], f32, tag="drcol")
                for jt in range(NJT):
                    ps = ps_mm.tile([P, JW], f32, tag="ps")
                    nc.tensor.matmul(
                        ps[:],
                        lhsT,
                        augy[:, jt * JW : (jt + 1) * JW],
                        start=True,
                        stop=True,
                    )
                    msl = M[:, jt * JW : (jt + 1) * JW]
                    if it == 0:
                        nc.scalar.activation(msl, ps[:], ACTF.Copy)
                        src = msl
                    else:
                        t = tpool.tile([P, JW], f16, tag="T")
                        nc.scalar.activation(t[:], ps[:], ACTF.Copy)
                        nc.vector.tensor_tensor(msl, t[:], msl, op=OP.min)
                        src = t[:]
                    if DR_MODE == "ts":
                        nc.vector.tensor_scalar(
                            src,
                            src,
                            BIG,
                            None,
                            op0=OP.min,
                            op1=OP.min,
                            accum_out=drcol[:, jt : jt + 1],
                        )
                    else:
                        nc.vector.tensor_tensor_reduce(
                            out=src,
                            in0=src,
                            in1=src,
                            scale=1.0,
                            scalar=1e30,
                            op0=OP.min,
                            op1=OP.min,
                            accum_out=drcol[:, jt : jt + 1],
                        )
                nc.vector.tensor_reduce(
                    DR[:, it : it + 1], drcol[:], axis=AX.X, op=OP.min
                )

            # ---- min over partitions (dl): PE transpose + free-dim min ----
            for k in range(NIT):
                pst = ps_tr.tile([P, P], f16, tag="pst")
                nc.tensor.transpose(pst[:], M[:, k * P : (k + 1) * P], ident_sb[:])
                nc.vector.tensor_reduce(
                    DL[:, k : k + 1], pst[:], axis=AX.X, op=OP.min
                )

            # ---- sums ----
            sm = accp.tile([P, 2], f32, tag="sm")
            nc.vector.tensor_reduce(sm[:, 0:1], DR[:], axis=AX.X, op=OP.add)
            nc.vector.tensor_reduce(sm[:, 1:2], DL[:], axis=AX.X, op=OP.add)
            sv = accp.tile([P, 1], f32, tag="sv")
            nc.vector.tensor_reduce(sv[:], sm[:], axis=AX.X, op=OP.add)
            psf = ps_fin.tile([1, 1], f32, tag="psf")
            nc.tensor.matmul(psf[:], sv[:], ones_col[:], start=True, stop=True)
            nc.scalar.activation(res[:, b : b + 1], psf[:], ACTF.Copy)

        outsb = resp.tile([1, 1], f32)
        nc.vector.tensor_reduce(outsb[:], res[:], axis=AX.X, op=OP.add)
        nc.sync.dma_start(out[:], outsb[:])

    if do_compile:
        nc.compile()
    return nc


def make_in_maps(preds, gts):
    ones = np.ones((1, N), np.float32)
    ident = np.eye(P, dtype=np.float16)
    in_maps = []
    for c in range(N_CORES):
        gb = gts[c * BPC : (c + 1) * BPC]  # x = gts
        pb = preds[c * BPC : (c + 1) * BPC]  # y = preds
        in_maps.append(
            {
                "xt": np.ascontiguousarray(gb.transpose(0, 2, 1)),
                "xr": np.ascontiguousarray(gb.reshape(BPC, NIT, D * P)),
                "yt": np.ascontiguousarray(pb.transpose(0, 2, 1)),
                "yr": np.ascontiguousarray(pb.reshape(BPC, NIT, D * P)),
                "ones": ones,
                "ident": ident,
            }
        )
    return in_maps


_prog = None
last_run_info = {}


def kernel(preds, gts):
    global _prog
    preds = np.ascontiguousarray(np.asarray(preds, dtype=np.float32))
    gts = np.ascontiguousarray(np.asarray(gts, dtype=np.float32))
    assert preds.shape == (B, N, D) and gts.shape == (B, N, D)
    if _prog is None:
        _prog = build_program()
    in_maps = make_in_maps(preds, gts)
    trace = bool(int(os.environ.get("CHAMFER_TRACE", "0")))
    r = run_bass_kernel_spmd(_prog, in_maps, list(range(N_CORES)), trace=trace)
    last_run_info["exec_time_ns"] = r.exec_time_ns
    last_run_info["results"] = r
    total = sum(float(m["out"][0, 0]) for m in r.results)
    return np.asarray(total / float(B * N), dtype=np.float32)
